# revision 66
# baseline (speedup 1.0000x reference)
"""AlchemicalGAT Trainium2 kernel (8 NeuronCores, SPMD).

Strategy:
  - Shard dst nodes contiguously across 8 cores (2500 each); edges sharded by dst.
  - Node phases compute, per owned node, a gather-table row
    [(h_c[64] | e_src_c)*4 | pad] (320 f32) plus an e_dst row table (64 f32),
    AllGather'd across cores into per-core HBM tables.
  - Edge phase per 128-dst-node block: dma_gather rows by src (h, e_src) and by
    dst (e_dst); softmax without max-subtraction (logits are small);
    1/den factored out of the message sum; segment-sum via one-hot matmul
    accumulating in PSUM over the block's edge tiles.
  - MLP + channel sum + per-structure matmul-masked segment sum on device;
    host sums the 8 per-core partial energy vectors.
"""
import sys, os
sys.path.insert(0, "/opt/trn_rl_repo")
import math
import numpy as np
from contextlib import ExitStack

import concourse.bass as bass
import concourse.bacc as bacc
import concourse.mybir as mybir
import concourse.tile as tile
from concourse.bass_utils import run_bass_kernel_spmd
from concourse.masks import make_identity

F32 = mybir.dt.float32
BF16 = mybir.dt.bfloat16
I16 = mybir.dt.int16
AF = mybir.ActivationFunctionType
ALU = mybir.AluOpType

N = 20000
E = 400000
C = 4
F = 128
O = 64          # conv1/conv2 width
H1, H2 = 64, 32
S = 32          # structures
NCORE = 8
NPC = N // NCORE            # 2500 nodes per core
BLK = 128
NBLK = math.ceil(NPC / BLK)  # 20 (last block 68 nodes)
LAST = NPC - (NBLK - 1) * BLK  # 68
PAD = N                      # pad row index
ROW = 384                    # gather-table row length (bf16, 256B-multiple)
ROWP = 264                   # packed row length actually exchanged/used
ROW2 = 256                   # e_dst local table row length (bf16): cols 0:C are
                             # e_dst, cols 128:256 the dst one-hot row (segsum
                             # lhsT comes straight out of the gather)
NEG = -1.0e30


def _set_sizes(n, npc):
    """Test hook: shrink the problem (n nodes total, npc per core)."""
    global N, NPC, NBLK, LAST, PAD
    N, NPC = n, npc
    NBLK = math.ceil(NPC / BLK)
    LAST = NPC - (NBLK - 1) * BLK
    PAD = N


# ---------------------------------------------------------------- host side

def _wrap_idx16(ids):
    """[n] int -> dma_gather index layout [128, n//16] int16 (q -> [q%16, q//16],
    replicated over the 8 Q7 cores)."""
    n = ids.shape[0]
    assert n % 16 == 0
    out = np.zeros((16, n // 16), np.int16)
    q = np.arange(n)
    out[q % 16, q // 16] = ids.astype(np.int16)
    return np.tile(out, (8, 1))


def _prep(inputs):
    x = np.ascontiguousarray(np.asarray(inputs["x"], dtype=np.float32))
    ei = np.asarray(inputs["edge_index"]).astype(np.int64)
    bid = np.asarray(inputs["batch_ids"]).astype(np.int64)
    gamma = np.asarray(inputs["gamma"], np.float32)
    beta = np.asarray(inputs["beta"], np.float32)
    src, dst = ei[0], ei[1]

    order = np.argsort(dst, kind="stable")
    src_s, dst_s = src[order], dst[order]

    # per (core, block) edge lists
    counts = np.bincount(dst_s, minlength=N)
    starts = np.concatenate([[0], np.cumsum(counts)])
    blk_cnt = np.zeros((NCORE, NBLK), np.int64)
    for c in range(NCORE):
        for b in range(NBLK):
            lo = c * NPC + b * BLK
            hi = min(c * NPC + min((b + 1) * BLK, NPC), (c + 1) * NPC)
            blk_cnt[c, b] = starts[hi] - starts[lo]
    tmax = int(math.ceil(blk_cnt.max() / BLK))
    EB = tmax * BLK  # padded edges per block

    # table row index: node n lives at (n//NPC)*(NPC+1) + n%NPC; row NPC of
    # core 0's slot is the pad row (the AllGather'd table carries one pad row
    # per core so the Shared output has a single writer).
    def tidx(n):
        return (n // NPC) * (NPC + 1) + n % NPC

    per_core = []
    for c in range(NCORE):
        src_ids = np.full((NBLK, EB), NPC, np.int64)
        dst_ids = np.full((NBLK, EB), NPC, np.int64)   # core-local (e_dst table)
        dloc = np.full((NBLK, EB), 127, np.int64)
        for b in range(NBLK):
            lo = c * NPC + b * BLK
            hi = min(c * NPC + min((b + 1) * BLK, NPC), (c + 1) * NPC)
            e0, e1 = starts[lo], starts[hi]
            n = e1 - e0
            src_ids[b, :n] = tidx(src_s[e0:e1])
            dst_ids[b, :n] = dst_s[e0:e1] - c * NPC
            dloc[b, :n] = dst_s[e0:e1] - lo
        # dma_gather index layout, per block side by side
        s16 = np.concatenate([_wrap_idx16(src_ids[b]) for b in range(NBLK)], axis=1)
        d16 = np.concatenate([_wrap_idx16(dst_ids[b]) for b in range(NBLK)], axis=1)
        # structure mask [128, NBLK*S]
        bm = np.zeros((128, NBLK * S), np.float32)
        for b in range(NBLK):
            cnt = BLK if b < NBLK - 1 else LAST
            g = c * NPC + b * BLK + np.arange(cnt)
            bm[np.arange(cnt), b * S + bid[g]] = 1.0
        xs = x[c * NPC:(c + 1) * NPC].reshape(NPC, C * F)
        per_core.append(dict(xs=xs, src16=s16, dst16=d16, bmask=bm))

    # weights
    Wc1 = np.asarray(inputs["Wc1"], np.float32) * gamma[None, :, None]
    Wc2 = np.asarray(inputs["Wc2"], np.float32)
    asrc1 = np.asarray(inputs["asrc1"], np.float32)
    adst1 = np.asarray(inputs["adst1"], np.float32)
    asrc2 = np.asarray(inputs["asrc2"], np.float32)
    adst2 = np.asarray(inputs["adst2"], np.float32)

    def wcat(W, a_s, a_d):
        out = np.zeros((C, W.shape[1], 66), np.float32)
        out[:, :, :O] = W
        out[:, :, O] = np.einsum("cfo,co->cf", W, a_s)
        out[:, :, O + 1] = np.einsum("cfo,co->cf", W, a_d)
        return out

    wcat1 = wcat(Wc1, asrc1, adst1)
    wcat2 = wcat(Wc2, asrc2, adst2)
    # beta fold: hL_T += (beta/gamma) per-f after LN scale (exact when gamma!=0)
    bg = beta / np.where(gamma == 0, 1.0, gamma)
    import ml_dtypes
    padrow = np.zeros((1, ROW), ml_dtypes.bfloat16)
    for cc in range(C):
        padrow[0, cc * 65 + 64] = NEG
    bf = ml_dtypes.bfloat16
    shared = dict(
        wcat1=wcat1.astype(bf), wcat2=wcat2.astype(bf),
        wn1=np.asarray(inputs["Wn1"], np.float32).astype(bf),
        wn2=np.asarray(inputs["Wn2"], np.float32).astype(bf),
        wout=(np.asarray(inputs["Wout"], np.float32)
              / np.float32(math.sqrt(C) * 20.0)).astype(bf),
        bg=np.tile(bg.reshape(128, 1), (1, 1)).astype(np.float32),
        padrow=padrow, zrow=np.zeros((1, ROW2), ml_dtypes.bfloat16),
    )
    return per_core, shared, tmax


# ---------------------------------------------------------------- device side

SHARED_TBL = os.environ.get("GAT_SHARED_TBL", "0") == "1"
STOP_AFTER = os.environ.get("GAT_STOP_AFTER", "")  # node1|ag1|edge1|node2|edge2


def _build(tmax):
    nc = bacc.Bacc("TRN2", target_bir_lowering=False, debug=False,
                   enable_asserts=False, num_devices=NCORE)
    TT = NBLK * tmax
    EB = tmax * BLK

    xs_d = nc.dram_tensor("xs", [NPC, C * F], F32, kind="ExternalInput")
    s16_d = nc.dram_tensor("src16", [128, TT * 8], I16, kind="ExternalInput")
    d16_d = nc.dram_tensor("dst16", [128, TT * 8], I16, kind="ExternalInput")
    bm_d = nc.dram_tensor("bmask", [128, NBLK * S], F32, kind="ExternalInput")
    wcat1_d = nc.dram_tensor("wcat1", [C, F, 66], BF16, kind="ExternalInput")
    wcat2_d = nc.dram_tensor("wcat2", [C, O, 66], BF16, kind="ExternalInput")
    wn1_d = nc.dram_tensor("wn1", [C, O, H1], BF16, kind="ExternalInput")
    wn2_d = nc.dram_tensor("wn2", [C, H1, H2], BF16, kind="ExternalInput")
    wout_d = nc.dram_tensor("wout", [C, H2, 1], BF16, kind="ExternalInput")
    bg_d = nc.dram_tensor("bg", [128, 1], F32, kind="ExternalInput")
    pr_d = nc.dram_tensor("padrow", [1, ROW], BF16, kind="ExternalInput")
    zr_d = nc.dram_tensor("zrow", [1, ROW2], BF16, kind="ExternalInput")
    out_d = nc.dram_tensor("energy", [S, 1], F32, kind="ExternalOutput")

    with tile.TileContext(nc, num_cores=NCORE) as tc, ExitStack() as ctx:
        per = ctx.enter_context(tc.tile_pool(name="persist", bufs=1))
        _wb = int(os.environ.get("GAT_SBW_BUFS", "3"))
        sbw = ctx.enter_context(tc.tile_pool(name="work", bufs=_wb))
        dpool = ctx.enter_context(tc.tile_pool(
            name="dwork", bufs=int(os.environ.get("GAT_DW_BUFS", "3"))))
        sbn = ctx.enter_context(tc.tile_pool(
            name="nwork", bufs=int(os.environ.get("GAT_SBN_BUFS", "4"))))
        ps_node = ctx.enter_context(tc.tile_pool(
            name="psn", bufs=int(os.environ.get("GAT_PSN_BUFS", "2")), space="PSUM"))
        ps_tp = ctx.enter_context(tc.tile_pool(name="pst", bufs=2, space="PSUM"))
        ps_edge = ctx.enter_context(tc.tile_pool(
            name="pse", bufs=int(os.environ.get("GAT_PSE_BUFS", "2")), space="PSUM"))
        ps_mlp = ctx.enter_context(tc.tile_pool(name="psm", bufs=1, space="PSUM"))
        dram = ctx.enter_context(tc.tile_pool(name="dram", bufs=1, space="DRAM"))

        # persistent tiles
        s16 = per.tile([128, TT * 8], I16)
        d16 = per.tile([128, TT * 8], I16)
        bm = per.tile([128, NBLK * S], F32)
        bg = per.tile([128, 1], F32)
        w1 = per.tile([F, C * 66], BF16)
        w2 = per.tile([O, C * 66], BF16)
        wn1 = per.tile([O, C * H1], BF16)
        wn2 = per.tile([H1, C * H2], BF16)
        wo = per.tile([H2, C], BF16)
        ident = per.tile([128, 128], F32)
        idb = per.tile([128, 128], BF16)
        Hb = per.tile([128, NBLK, C * O], BF16)
        H3 = per.tile([128, NBLK, C * O], BF16)
        Eb = per.tile([128, NBLK], F32)
        eps = per.tile([128, 1], F32)
        nc.vector.memset(eps[:], 1e-5)
        half = per.tile([128, 1], F32)
        nc.vector.memset(half[:], 0.5)

        nc.sync.dma_start(s16[:], s16_d[:, :])
        nc.sync.dma_start(d16[:], d16_d[:, :])
        nc.sync.dma_start(bm[:], bm_d[:, :])
        nc.sync.dma_start(bg[:], bg_d[:, :])
        for cc in range(C):
            nc.sync.dma_start(w1[:, cc * 66:(cc + 1) * 66], wcat1_d[cc])
            nc.sync.dma_start(w2[:, cc * 66:(cc + 1) * 66], wcat2_d[cc])
            nc.sync.dma_start(wn1[:, cc * H1:(cc + 1) * H1], wn1_d[cc])
            nc.sync.dma_start(wn2[:, cc * H2:(cc + 1) * H2], wn2_d[cc])
            nc.sync.dma_start(wo[:, cc:cc + 1], wout_d[cc])
        make_identity(nc, ident[:])
        nc.vector.tensor_copy(idb[:], ident[:])

        # DRAM tables
        tbsrc = [dram.tile([NPC + 1, ROW], BF16, name=f"tbsrc{i}") for i in range(2)]
        # e_dst table is core-local (dst nodes are always owned) — no AllGather
        t2src = [dram.tile([NPC + 1, ROW2], BF16, name=f"t2src{i}") for i in range(2)]
        tbl = [dram.tile([N + NCORE, ROW], BF16, name=f"tbl{i}")
               for i in range(2)]

        def silu_to(pool, out_ap, in_ap, shape, tag):
            """out = in * sigmoid(in) via tanh (ACT exp-set friendly);
            the 0.5*t+0.5 affine also runs on ACT (Identity) to spare DVE."""
            th = pool.tile(shape, F32, tag=tag)
            nc.scalar.activation(th[:], in_ap, AF.Tanh, scale=0.5)
            nc.vector.tensor_scalar(out=th[:], in0=th[:], scalar1=0.5, scalar2=0.5,
                                    op0=ALU.mult, op1=ALU.add)
            nc.vector.tensor_tensor(out=out_ap, in0=in_ap, in1=th[:], op=ALU.mult)

        def node_block(layer, nt):
            """layer 0: from xs (LN + Wcat1); layer 1: from Hb (Wcat2)."""
            wk, kdim = (w1, F) if layer == 0 else (w2, O)
            if True:
                cnt = BLK if nt < NBLK - 1 else LAST
                nps = ps_node.tile([128, C * 66], F32, space="PSUM", tag="nps")
                if layer == 0:
                    # channel-batched LayerNorm on [128, C, F]
                    xt = sbn.tile([128, C * F], F32, tag="xt")
                    if nt == NBLK - 1:
                        nc.vector.memset(xt[:], 0.0)
                    nc.sync.dma_start(xt[:cnt], xs_d[nt * BLK:nt * BLK + cnt, :])
                    xv = xt[:, :].rearrange("p (c f) -> p c f", c=C)
                    nm = sbn.tile([128, C, 1], F32, tag="nm")
                    nc.vector.tensor_reduce(out=nm[:], in_=xv,
                                            axis=mybir.AxisListType.X, op=ALU.add)
                    nc.vector.tensor_scalar_mul(nm[:], nm[:], -1.0 / F)
                    # var = E[x^2] - mu^2 from RAW x (safe: mu~0, var~1 here);
                    # avoids materializing x-mu
                    ss = sbn.tile([128, C, 1], F32, tag="ss")
                    for cc in range(C):
                        sq = sbn.tile([128, F], F32, tag="sq")
                        nc.scalar.activation(sq[:], xt[:, cc * F:(cc + 1) * F],
                                             AF.Square, accum_out=ss[:, cc, :])
                    nc.vector.tensor_scalar_mul(ss[:], ss[:], 1.0 / F)
                    mu2 = sbn.tile([128, C, 1], F32, tag="mu2")
                    nc.vector.tensor_tensor(out=mu2[:], in0=nm[:], in1=nm[:],
                                            op=ALU.mult)
                    nc.vector.tensor_tensor(out=ss[:], in0=ss[:], in1=mu2[:],
                                            op=ALU.subtract)
                    sg = sbn.tile([128, C], F32, tag="sg")
                    nc.scalar.activation(sg[:], ss[:, :, 0], AF.Sqrt, bias=eps[:])
                    rr = sbn.tile([128, C, 1], F32, tag="rr")
                    nc.vector.reciprocal(rr[:, :, 0], sg[:])
                    hl = sbn.tile([128, C, F], BF16, tag="hl")
                    for cc in range(C):
                        # hl_c = (x_c + nm_c) * rr_c, fused per-partition scalars
                        nc.vector.tensor_scalar(
                            out=hl[:, cc, :], in0=xt[:, cc * F:(cc + 1) * F],
                            scalar1=nm[:, cc, :], scalar2=rr[:, cc, :],
                            op0=ALU.add, op1=ALU.mult)
                    tp4 = ps_tp.tile([128, C * 128], BF16, space="PSUM", tag="tp")
                    for cc in range(C):
                        nc.tensor.transpose(out=tp4[:, cc * 128:(cc + 1) * 128],
                                            in_=hl[:, cc, :], identity=idb[:])
                    hlt4 = sbn.tile([128, C * 128], BF16, tag="hlt")
                    nc.vector.tensor_scalar(out=hlt4[:], in0=tp4[:], scalar1=bg[:],
                                            scalar2=None, op0=ALU.add)
                else:
                    tp4 = ps_tp.tile([O, C * 128], BF16, space="PSUM", tag="tp")
                    for cc in range(C):
                        nc.tensor.transpose(out=tp4[:, cc * 128:(cc + 1) * 128],
                                            in_=Hb[:, nt, cc * O:(cc + 1) * O],
                                            identity=idb[:])
                    hlt4 = sbn.tile([O, C * 128], BF16, tag="hlt")
                    nc.scalar.activation(hlt4[:], tp4[:], AF.Identity)
                for cc in range(C):
                    nc.tensor.matmul(out=nps[:, cc * 66:(cc + 1) * 66],
                                     lhsT=hlt4[:kdim, cc * 128:(cc + 1) * 128],
                                     rhs=wk[:kdim, cc * 66:(cc + 1) * 66],
                                     start=True, stop=True)
                # assemble table tiles (bf16; e_src/e_dst plain bf16; the
                # t2 row carries the node's block-local one-hot in 128:256
                # so the edge phase gets its segsum lhsT from the D-gather)
                tt = sbn.tile([128, ROW], BF16, tag="tt")
                t2t = sbn.tile([128, ROW2], BF16, tag="t2t")
                nc.vector.memset(tt[:, 260:ROW], 0.0)
                nc.vector.memset(t2t[:, C:128], 0.0)
                nc.vector.tensor_copy(t2t[:, 128:ROW2], idb[:, :])
                npsv = nps[:, :].rearrange("p (c u) -> p c u", c=C)
                ttv = tt[:, 0:260].rearrange("p (c u) -> p c u", c=C)
                if layer == 0:
                    # node1 runs standalone: DVE has slack, ACT is its ceiling
                    nc.vector.tensor_copy(ttv[:, :, 0:65], npsv[:, :, 0:65])
                else:
                    # node2 interleaves with edge1 where DVE is saturated
                    nc.scalar.activation(ttv[:, :, 0:65], npsv[:, :, 0:65],
                                         AF.Identity)
                nc.scalar.activation(t2t[:, 0:C], npsv[:, :, 65], AF.Identity)
                nc.sync.dma_start(tbsrc[layer][nt * BLK:nt * BLK + cnt, :], tt[:cnt])
                nc.sync.dma_start(t2src[layer][nt * BLK:nt * BLK + cnt, :], t2t[:cnt])

        def node_finish(layer):
            # pad rows (row NPC of each core's contribution) + gather tables
            prt = sbn.tile([1, ROW], BF16, tag="prt")
            zrt = sbn.tile([1, ROW2], BF16, tag="zrt")
            nc.sync.dma_start(prt[:], pr_d[:, :])
            nc.sync.dma_start(zrt[:], zr_d[:, :])
            nc.sync.dma_start(tbsrc[layer][NPC:NPC + 1, :], prt[:])
            nc.sync.dma_start(t2src[layer][NPC:NPC + 1, :], zrt[:])
            nc.gpsimd.collective_compute(
                "AllGather", ALU.bypass, replica_groups=[list(range(NCORE))],
                ins=[tbsrc[layer][:, :].opt()], outs=[tbl[layer][:, :].opt()])

        # NOTE: >1024 idxs per dma_gather crashes the device — keep chunks at 8
        # tiles.
        gchunk = int(os.environ.get("GAT_GCHUNK", "1024")) // 128  # tiles per gather

        PF = 3  # D-gathers have no dep on the AllGather: issue the first few
                # blocks' (= all D pool slots) before any G-gather so they run
                # inside the collective window

        def gather_d(layer, b):
            D = dpool.tile([128, tmax, ROW2], BF16, tag="D")
            for t0 in range(0, tmax, gchunk):
                t1 = min(t0 + gchunk, tmax)
                nn = (t1 - t0) * 128
                o0 = b * tmax * 8 + t0 * 8
                nc.gpsimd.dma_gather(D[:, t0:t1, :], t2src[layer][:, :],
                                     d16[:, o0:o0 + (t1 - t0) * 8], nn, nn, ROW2)
            return D

        def edge_phase(layer, Hout, post_block=None):
            dpre = [gather_d(layer, b) for b in range(PF)]
            for b in range(NBLK):
                G = sbw.tile([128, tmax, ROW], BF16, tag="G")
                D = dpre[b] if b < PF else gather_d(layer, b)
                for t0 in range(0, tmax, gchunk):
                    t1 = min(t0 + gchunk, tmax)
                    nn = (t1 - t0) * 128
                    o0 = b * tmax * 8 + t0 * 8
                    nc.gpsimd.dma_gather(G[:, t0:t1, :], tbl[layer][:, :],
                                         s16[:, o0:o0 + (t1 - t0) * 8], nn, nn, ROW)
                Gv = G[:, :, 0:260].rearrange("p t (c u) -> p t c u", c=C)
                EX = sbw.tile([128, tmax, C], F32, tag="EX")
                LK = sbw.tile([128, tmax, C], F32, tag="LK")
                EXb = sbw.tile([128, tmax, C], BF16, tag="EXb")
                ps = ps_edge.tile([128, 260], F32, space="PSUM", tag="ep")
                # process per gather-chunk so compute on chunk k overlaps the
                # later chunks' gathers instead of waiting for the whole block
                for t0 in range(0, tmax, gchunk):
                    t1 = min(t0 + gchunk, tmax)
                    sl = slice(t0, t1)
                    w = t1 - t0
                    # logit = e_src + e_dst, leaky_relu(x) = max(x, 0.2x)
                    nc.vector.tensor_tensor(out=EX[:, sl], in0=Gv[:, sl, :, 64],
                                            in1=D[:, sl, 0:C], op=ALU.add)
                    nc.vector.tensor_scalar_mul(LK[:, sl], EX[:, sl], 0.2)
                    nc.vector.tensor_tensor(out=EX[:, sl], in0=EX[:, sl],
                                            in1=LK[:, sl], op=ALU.max)
                    # exp to bf16 (all-16-bit h*ex multiply)
                    nc.scalar.activation(EXb[:, sl], EX[:, sl], AF.Exp)
                    nc.vector.tensor_tensor(
                        out=Gv[:, sl, :, 0:64], in0=Gv[:, sl, :, 0:64],
                        in1=EXb[:, sl].to_broadcast([128, w, C, 64]), op=ALU.mult)
                    nc.vector.tensor_copy(Gv[:, sl, :, 64], EXb[:, sl])
                    for t in range(t0, t1):
                        # dst one-hot rides in the D-gather rows (cols 128:256)
                        nc.tensor.matmul(out=ps[:], lhsT=D[:, t, 128:256],
                                         rhs=G[:, t, 0:260],
                                         start=(t == 0), stop=(t == tmax - 1))
                psv = ps[:, :].rearrange("p (c u) -> p c u", c=C)
                dn = sbw.tile([128, C], F32, tag="dn")
                nc.vector.tensor_scalar(out=dn[:], in0=psv[:, :, 64], scalar1=1e-16,
                                        scalar2=None, op0=ALU.add)
                rc = sbw.tile([128, C], F32, tag="rc")
                nc.vector.reciprocal(rc[:], dn[:])
                om = sbw.tile([128, C * O], F32, tag="om")
                omv = om[:, :].rearrange("p (c u) -> p c u", c=C)
                nc.vector.tensor_tensor(out=omv[:, :, :], in0=psv[:, :, 0:64],
                                        in1=rc[:].to_broadcast([128, C, 64]), op=ALU.mult)
                silu_to(sbw, Hout[:, b, :], om[:], [128, C * O], "th")
                if post_block is not None:
                    post_block(b)

        phases = {"node1": 1, "ag1": 2, "edge1": 3, "node2": 4, "edge2": 5}
        stop = phases.get(STOP_AFTER, 99)

        for nt in range(NBLK):
            node_block(0, nt)
        if stop >= 2:
            node_finish(0)
        if stop >= 3:
            # node2's block work interleaves with edge1 so AG2 starts as soon
            # as edge1 drains (in-order SEQs would otherwise defer it all)
            edge_phase(0, Hb,
                       post_block=(lambda b: node_block(1, b)) if stop >= 4 else None)
        # MLP + channel sum — channel-batched over groups of QB node blocks
        # (QB*128 columns per matmul, bf16 operands); interleaved into edge2's
        # block loop so the tail drains with the last edge block
        QB = 4

        def tail_quarter(q):
            u2s = []
            for cc in range(C):
                psT = ps_mlp.tile([O, QB * 128], BF16, space="PSUM", tag="mlpT")
                for j in range(QB):
                    nt = q * QB + j
                    nc.tensor.transpose(out=psT[:, j * 128:(j + 1) * 128],
                                        in_=H3[:, nt, cc * O:(cc + 1) * O],
                                        identity=idb[:])
                u0 = sbn.tile([O, QB * 128], BF16, tag="u0")
                nc.vector.tensor_copy(u0[:], psT[:])
                p1 = ps_mlp.tile([H1, QB * 128], F32, space="PSUM", tag="pmm")
                nc.tensor.matmul(out=p1[:], lhsT=wn1[:, cc * H1:(cc + 1) * H1],
                                 rhs=u0[:], start=True, stop=True)
                u1 = sbn.tile([H1, QB * 128], BF16, tag="u1")
                silu_to(sbn, u1[:], p1[:], [H1, QB * 128], "th1")
                p2 = ps_mlp.tile([H2, QB * 128], F32, space="PSUM", tag="pmm")
                nc.tensor.matmul(out=p2[:], lhsT=wn2[:, cc * H2:(cc + 1) * H2],
                                 rhs=u1[:], start=True, stop=True)
                u2 = sbn.tile([H2, QB * 128], BF16, tag=f"u2_{cc}")
                silu_to(sbn, u2[:], p2[:], [H2, QB * 128], "th2")
                u2s.append(u2)
            for j in range(QB):
                nt = q * QB + j
                p3 = ps_tp.tile([128, 1], F32, space="PSUM", tag="tp")
                for cc in range(C):
                    nc.tensor.matmul(out=p3[:], lhsT=u2s[cc][:, j * 128:(j + 1) * 128],
                                     rhs=wo[:, cc:cc + 1],
                                     start=(cc == 0), stop=(cc == C - 1))
                nc.vector.tensor_copy(Eb[:, nt:nt + 1], p3[:])

        def edge2_post(b):
            if (b + 1) % QB == 0:
                tail_quarter((b + 1) // QB - 1)

        if stop >= 5:
            node_finish(1)
            edge_phase(1, H3, post_block=edge2_post)
        if stop < 6:
            eo0 = sbn.tile([S, 1], F32, tag="eo")
            nc.vector.memset(eo0[:], 0.0)
            nc.sync.dma_start(out_d[:, :], eo0[:])
            do_tail = False
        else:
            do_tail = True
        if do_tail:
            psS = ps_tp.tile([S, 1], F32, space="PSUM", tag="tp")
            for nt in range(NBLK):
                nc.tensor.matmul(out=psS[:], lhsT=bm[:, nt * S:(nt + 1) * S],
                                 rhs=Eb[:, nt:nt + 1],
                                 start=(nt == 0), stop=(nt == NBLK - 1))
            eo = sbn.tile([S, 1], F32, tag="eo")
            nc.vector.tensor_copy(eo[:], psS[:])
            nc.sync.dma_start(out_d[:, :], eo[:])
    nc.compile()
    return nc


_CACHE = {}


def kernel(**inputs):
    per_core, shared, tmax = _prep(inputs)
    if tmax not in _CACHE:
        _CACHE[tmax] = _build(tmax)
    nc = _CACHE[tmax]
    in_maps = []
    for c in range(NCORE):
        pc = per_core[c]
        in_maps.append({
            "xs": pc["xs"], "src16": pc["src16"], "dst16": pc["dst16"],
            "bmask": pc["bmask"],
            "wcat1": shared["wcat1"], "wcat2": shared["wcat2"],
            "wn1": shared["wn1"], "wn2": shared["wn2"], "wout": shared["wout"],
            "bg": shared["bg"], "padrow": shared["padrow"], "zrow": shared["zrow"],
        })
    res = run_bass_kernel_spmd(nc, in_maps, core_ids=list(range(NCORE)))
    out = np.zeros((S, 1), np.float32)
    for c in range(NCORE):
        out += res.results[c]["energy"]
    return out

